# revision 6
# baseline (speedup 1.0000x reference)
"""CoAttention kernel for 8x TRN2 NeuronCores.

Computation (per batch b):
    q = x[b] @ Wq.T + bq            [Sq, H]
    k = y[b] @ Wk.T + bk            [Skv, H]
    v = y[b] @ Wv.T + bv            [Skv, H]
    out[b] = softmax(q @ k.T / sqrt(H)) @ v

Sharding: data-parallel over batch; each of the 8 cores handles B/8 = 2
batches. Weights are replicated. Host staging transposes activations to
[D, S] (contraction dim on partitions) and casts matmul operands to fp16
(PE runs fp16 at 4x the fp32 rate; fp32 accumulation in PSUM keeps the
absmax-relative error ~4e-4, verified against a float64 reference).

Device-side layout choices:
  - Q^T [H, Sq] and K^T [H, Skv] (H on partitions) so the score matmul
    contracts over H, and the per-partition bias add is free on DVE.
  - Scores are built TRANSPOSED: S^T[t, s] = (K^T tile).T @ Q^T, so that
    P^T = exp(S^T) is directly usable as the stationary operand of the
    P @ V matmul (contraction over t on partitions).
  - Softmax denominator comes for free as a ones-column appended to V:
    out_psum[:, H] = sum_t P^T[t, s]. No max-subtraction is needed:
    logits are O(1) here, exp cannot overflow, and softmax is shift-
    invariant so the result matches the reference exactly.
  - bv is folded past the softmax: rows of softmax sum to 1, so
    out = (P @ v_raw) / denom + bv.
"""

import os
import sys
from contextlib import ExitStack

import numpy as np

sys.path.insert(0, "/opt/trn_rl_repo")

N_CORES = 8
B, SQ, SKV, D, H = 16, 1024, 1024, 768, 256
BL = B // N_CORES  # batches per core
KD = D // 128      # 6 contraction tiles for the projections
JH = H // 128      # 2 partition tiles of hidden
TS = SKV // 128    # 8 kv tiles
SB = SQ // 512     # 2 query blocks of 512

_cached = {}


def _build_nc(reps=1):
    import concourse.bass as bass
    import concourse.tile as tile
    from concourse import bacc, mybir

    f16 = mybir.dt.float16
    f32 = mybir.dt.float32
    Exp = mybir.ActivationFunctionType.Exp
    Copy = mybir.ActivationFunctionType.Copy
    mult = mybir.AluOpType.mult
    add = mybir.AluOpType.add

    nc = bacc.Bacc("TRN2", target_bir_lowering=False, debug=False)

    xT = nc.dram_tensor("xT", [BL, D, SQ], f16, kind="ExternalInput")
    yT = nc.dram_tensor("yT", [BL, D, SKV], f16, kind="ExternalInput")
    wqT = nc.dram_tensor("wqT", [D, H], f16, kind="ExternalInput")
    wkT = nc.dram_tensor("wkT", [D, H], f16, kind="ExternalInput")
    wvT = nc.dram_tensor("wvT", [D, H], f16, kind="ExternalInput")
    bqd = nc.dram_tensor("bqd", [JH, 128, 1], f32, kind="ExternalInput")
    bkd = nc.dram_tensor("bkd", [JH, 128, 1], f32, kind="ExternalInput")
    bvd = nc.dram_tensor("bvd", [128, H], f32, kind="ExternalInput")
    outd = nc.dram_tensor("out", [BL, SQ, H], f32, kind="ExternalOutput")

    with tile.TileContext(nc) as tc, ExitStack() as ctx:
        wpool = ctx.enter_context(tc.tile_pool(name="w", bufs=1))
        cpool = ctx.enter_context(tc.tile_pool(name="c", bufs=1))
        xpool = ctx.enter_context(tc.tile_pool(name="acts", bufs=2))
        qkv = ctx.enter_context(tc.tile_pool(name="qkv", bufs=2))
        ptp = ctx.enter_context(tc.tile_pool(name="ptp", bufs=4))
        outp = ctx.enter_context(tc.tile_pool(name="outp", bufs=4))
        smallp = ctx.enter_context(tc.tile_pool(name="small", bufs=4))
        psA = ctx.enter_context(
            tc.tile_pool(name="psA", bufs=2, space=bass.MemorySpace.PSUM)
        )
        psS = ctx.enter_context(
            tc.tile_pool(name="psS", bufs=2, space=bass.MemorySpace.PSUM)
        )
        psO = ctx.enter_context(
            tc.tile_pool(name="psO", bufs=4, space=bass.MemorySpace.PSUM)
        )

        # Replicated constants
        wq_sb = wpool.tile([128, KD, H], f16, tag="wq")
        nc.sync.dma_start(wq_sb[:], wqT[:].rearrange("(k p) h -> p k h", p=128))
        wk_sb = wpool.tile([128, KD, H], f16, tag="wk")
        nc.sync.dma_start(wk_sb[:], wkT[:].rearrange("(k p) h -> p k h", p=128))
        wv_sb = wpool.tile([128, KD, H], f16, tag="wv")
        nc.sync.dma_start(wv_sb[:], wvT[:].rearrange("(k p) h -> p k h", p=128))
        bq_sb = cpool.tile([128, JH], f32, tag="bq")
        bk_sb = cpool.tile([128, JH], f32, tag="bk")
        for j in range(JH):
            nc.sync.dma_start(bq_sb[:, j : j + 1], bqd[j])
            nc.sync.dma_start(bk_sb[:, j : j + 1], bkd[j])
        bv_sb = cpool.tile([128, H], f32, tag="bv")
        nc.sync.dma_start(bv_sb[:], bvd[:])

        def emit_body():
            for b in range(BL):
                emit_batch(b)

        def emit_batch(b):
            # Activations, one tile per contraction slice so matmuls can
            # start as soon as the first slice lands.
            xts, yts = [], []
            for k in range(KD):
                xt = xpool.tile([128, SQ], f16, tag=f"xt{k}", name=f"xt{k}_{b}")
                nc.sync.dma_start(xt[:], xT[b, 128 * k : 128 * (k + 1), :])
                xts.append(xt)
            for k in range(KD):
                yt = xpool.tile([128, SKV], f16, tag=f"yt{k}", name=f"yt{k}_{b}")
                nc.sync.dma_start(yt[:], yT[b, 128 * k : 128 * (k + 1), :])
                yts.append(yt)

            qt_sb = qkv.tile([128, JH, SQ], f16, tag="qt", name=f"qt_{b}")
            kt_sb = qkv.tile([128, JH, SKV], f16, tag="kt", name=f"kt_{b}")
            v_sb = qkv.tile([128, TS, H + 1], f16, tag="v", name=f"v_{b}")

            # Q^T / K^T projections: psum[h, s_half] += WxT_k.T @ actT_k
            for w_sb, acts, bias_sb, dst in (
                (wq_sb, xts, bq_sb, qt_sb),
                (wk_sb, yts, bk_sb, kt_sb),
            ):
                for j in range(JH):
                    for hv in range(2):
                        pp = psA.tile([128, 512], f32, tag="proj", name=f"pp{b}")
                        for k in range(KD):
                            nc.tensor.matmul(
                                pp[:],
                                w_sb[:, k, 128 * j : 128 * (j + 1)],
                                acts[k][:, 512 * hv : 512 * (hv + 1)],
                                start=(k == 0),
                                stop=(k == KD - 1),
                            )
                        nc.vector.tensor_scalar_add(
                            dst[:, j, 512 * hv : 512 * (hv + 1)],
                            pp[:],
                            bias_sb[:, j : j + 1],
                        )

            # V projection (no bias; folded into the epilogue): V[t, h]
            for t in range(TS):
                pv = psA.tile([128, H], f32, tag="proj", name=f"pv{b}")
                for k in range(KD):
                    nc.tensor.matmul(
                        pv[:],
                        yts[k][:, 128 * t : 128 * (t + 1)],
                        wv_sb[:, k, :],
                        start=(k == 0),
                        stop=(k == KD - 1),
                    )
                nc.scalar.activation(v_sb[:, t, 0:H], pv[:], Copy)
                nc.vector.memset(v_sb[:, t, H : H + 1], 1.0)

            # Attention, one 512-query block at a time
            for sb in range(SB):
                ops = [
                    psO.tile([128, H + 1], f32, tag="o", name=f"op{b}_{sb}_{j}")
                    for j in range(4)
                ]
                for t in range(TS):
                    st = psS.tile([128, 512], f32, tag="st", name=f"st{b}")
                    for j2 in range(JH):
                        nc.tensor.matmul(
                            st[:],
                            kt_sb[:, j2, 128 * t : 128 * (t + 1)],
                            qt_sb[:, j2, 512 * sb : 512 * (sb + 1)],
                            start=(j2 == 0),
                            stop=(j2 == JH - 1),
                        )
                    pt = ptp.tile([128, 512], f16, tag="pt", name=f"pt{b}")
                    nc.scalar.activation(pt[:], st[:], Exp, scale=float(H) ** -0.5)
                    for j in range(4):
                        nc.tensor.matmul(
                            ops[j][:],
                            pt[:, 128 * j : 128 * (j + 1)],
                            v_sb[:, t, :],
                            start=(t == 0),
                            stop=(t == TS - 1),
                        )
                # epilogue: out = pv / denom + bv
                for j in range(4):
                    si = 4 * sb + j
                    rec = smallp.tile([128, 1], f32, tag="rec", name=f"rec{b}")
                    nc.vector.reciprocal(rec[:], ops[j][:, H : H + 1])
                    ot = outp.tile([128, H], f32, tag="ot", name=f"ot{b}")
                    nc.vector.scalar_tensor_tensor(
                        ot[:], ops[j][:, 0:H], rec[:], bv_sb[:], op0=mult, op1=add
                    )
                    nc.sync.dma_start(
                        outd[b, 128 * si : 128 * (si + 1), :], ot[:]
                    )

        if reps == 1:
            emit_body()
        else:
            # Device-side repetition for wall-clock benchmarking (the
            # per-call dispatch overhead through axon is ~80ms, far above
            # the kernel's span; the R-vs-1 slope isolates HW time).
            with tc.For_i(0, reps, 1):
                emit_body()

    nc.compile()
    return nc


def _get_nc(reps=1):
    key = ("nc", reps)
    if key not in _cached:
        _cached[key] = _build_nc(reps)
    return _cached[key]


def make_in_maps(x, y, Wq, bq, Wk, bk, Wv, bv):

    f16 = np.float16
    wq_h = np.ascontiguousarray(Wq.T).astype(f16)  # [D, H]
    wk_h = np.ascontiguousarray(Wk.T).astype(f16)
    wv_h = np.ascontiguousarray(Wv.T).astype(f16)
    bq_h = np.ascontiguousarray(bq.reshape(JH, 128, 1)).astype(np.float32)
    bk_h = np.ascontiguousarray(bk.reshape(JH, 128, 1)).astype(np.float32)
    bv_h = np.ascontiguousarray(
        np.broadcast_to(bv.astype(np.float32), (128, H))
    )

    in_maps = []
    for c in range(N_CORES):
        xs = np.asarray(x[BL * c : BL * (c + 1)])  # [BL, Sq, D]
        ys = np.asarray(y[BL * c : BL * (c + 1)])
        in_maps.append(
            {
                "xT": np.ascontiguousarray(xs.transpose(0, 2, 1)).astype(f16),
                "yT": np.ascontiguousarray(ys.transpose(0, 2, 1)).astype(f16),
                "wqT": wq_h,
                "wkT": wk_h,
                "wvT": wv_h,
                "bqd": bq_h,
                "bkd": bk_h,
                "bvd": bv_h,
            }
        )
    return in_maps


def kernel(x, y, Wq, bq, Wk, bk, Wv, bv):
    from concourse.bass_utils import run_bass_kernel_spmd

    nc = _get_nc()
    in_maps = make_in_maps(x, y, Wq, bq, Wk, bk, Wv, bv)
    bkr = run_bass_kernel_spmd(
        nc,
        in_maps,
        list(range(N_CORES)),
        trace=bool(os.environ.get("KERNEL_TRACE")),
    )
    _cached["last_results"] = bkr
    return np.concatenate([r["out"] for r in bkr.results], axis=0)
